# revision 22
# baseline (speedup 1.0000x reference)
"""PointNet feature extractor on 8 Trainium2 NeuronCores (Bass/Tile), v7.

Problem: x (8, 16384, 3) -> 3x [conv1d(k=1) + sync-BN (+ReLU)] ->
global max-pool -> out (8, 1088, 16384) where rows 0:1024 are the
broadcast global feature and rows 1024:1088 are the (transposed) local
(layer-0) features.

Structure (v7):
  * AR0 eliminated: x is replicated into every core's packed input (a
    sharding choice -- inputs are device-resident), so each core
    computes the GLOBAL layer-0 BN stats locally from all 8 batches:
    9 DVE reductions give the x second moments; host-packed
    pair-product weights (Winograd-style weight transform) turn them
    into sumsq(u0)/sum(u0) with 4 tiny matmuls.
  * Layer-2 BN stats via the Gram trick: sumsq(h2) = diag(W2 G W2^T)
    with G = sum_n h1 h1^T, so the layer-2 AllReduce launches BEFORE
    the big [128->1024] matmuls and hides under them; no ACT Square
    passes at all.
  * h1^T (for G) from PE transposes trailing the relu1 chunks.
  * Per-block pipeline: 128-channel block matmuls -> DVE max -> BN
    affine -> 8MB broadcast write, writes streaming back-to-back on
    the SP HWDGE ring, overlapped with block compute and the BN2
    AllReduce.
  * Layer 0/1 matmuls in float32r (full-rate fp32); h0 kept f32 for
    accuracy; bf16 for layer-2 matmuls and h1/h1^T.

Key algebraic facts used:
  * conv biases b0/b1/b2 cancel inside training-mode BN.
  * BN scale is positive (gamma=1), so max_n BN(u) = BN(max_n u).
  * mean of h2 = W2 @ (AllReduce sum of h1) / N_total.
"""

import functools
import numpy as np

B = 8
N = 16384          # points per batch == points per core (1 batch / core)
NTOT = B * N       # BN statistics population size
EPS = 1e-5
NCORES = 8
F32R = True        # float32r (full-rate fp32) for layer-1 matmuls
NO_TRANSPOSE = False   # debug: skip h1T transposes + G2 (wrong stats, crash-test)
BC_ON_DVE = False      # debug: broadcast fills on DVE instead of ACT
LOCAL_SIMPLE = False   # debug: local write as plain contiguous DMA
STAGE = 5              # debug: 1=stats 2=+L0 3=+L1/AR1 4=+relu1/AR2 5=full
TTR_OFF = False        # debug: replace tensor_tensor_reduce with mul+reduce

# ---- packed input column layout (f32 words) ----
XALL = 0             # [128, 3072] all-batch x, component-major (3 x 1024 cols)
XF = 3072            # [128, 8192] own-batch xT: rows 0:3 = points 0:8192,
                     #                          rows 64:67 = points 8192:16384
W1T2 = XF + 8192     # [128, 128] f32 W1^T stacked twice (partition-fold combine)
I128B = W1T2 + 128   # [128, 64] words = [128, 128] bf16 identity
W2TB = I128B + 64    # [128, 512] words = [128, 1024] bf16 W2^T
W2N = W2TB + 512     # [128, 1024] f32 W2 natural (block-major)
W0R = W2N + 1024     # [128, 64] f32 W0^T at partition rows 0:3 and 64:67
WPA = W0R + 64       # [9(->128), 64] f32 pair-product weights (sumsq(u0))
WPB = WPA + 64       # [9, 64] f32 mean weights (sum(u0))
IREP = WPB + 64      # [64, 128] f32 [I64 | I64] partition-replication matrix
SMALL = IREP + 128   # col 0 g0rep, 1 be0rep, 2 g1, 3 be1, 4:12 g2blk, 12:20 be2blk
WTOT = SMALL + 20


def _body(nc, tc, io):
    import concourse.bass as bass
    from concourse import mybir

    f32 = mybir.dt.float32
    f32r = mybir.dt.float32r
    bf16 = mybir.dt.bfloat16
    AF = mybir.ActivationFunctionType
    OP = mybir.AluOpType
    AX = mybir.AxisListType
    RG = [list(range(NCORES))]

    out = io["out"]
    allf = io["allf"]

    with (
        tc.tile_pool(name="wgt", bufs=1) as wgt,
        tc.tile_pool(name="big", bufs=1) as big,
        tc.tile_pool(name="scr", bufs=2) as scr,
        tc.tile_pool(name="bcp", bufs=2) as bcp,
        tc.tile_pool(name="stat", bufs=1) as stat,
        tc.tile_pool(name="dram", bufs=1, space="DRAM") as dram,
    ):
        # ---------------- input loads ----------------
        sb_pk = wgt.tile([128, WTOT - W1T2], f32)
        nc.scalar.dma_start(sb_pk[:], allf[:, W1T2:WTOT])
        o = -W1T2
        sb_w1t2 = sb_pk[:, W1T2 + o:I128B + o]
        sb_i128b = sb_pk[:, I128B + o:W2TB + o].bitcast(bf16)
        sb_w2tb = sb_pk[:, W2TB + o:W2N + o].bitcast(bf16)
        sb_w2n = sb_pk[:, W2N + o:W0R + o]
        sb_w0r = sb_pk[:, W0R + o:WPA + o]
        sb_wpa = sb_pk[0:9, WPA + o:WPB + o]
        sb_wpb = sb_pk[0:9, WPB + o:IREP + o]
        sb_irep = sb_pk[0:64, IREP + o:SMALL + o]
        sb_g0 = sb_pk[:, SMALL + o:SMALL + o + 2]
        sb_g1 = sb_pk[:, SMALL + o + 2:SMALL + o + 4]
        sb_g2 = sb_pk[:, SMALL + o + 4:SMALL + o + 12]
        sb_be2 = sb_pk[:, SMALL + o + 12:SMALL + o + 20]

        sb_xall = big.tile([128, 3072], f32, tag="A")
        nc.sync.dma_start(sb_xall[:], allf[:, XALL:XALL + 3072])
        mdt = f32r if F32R else f32
        def mcast(ap):
            return ap.bitcast(f32r) if F32R else ap
        sb_xf = big.tile([128, 8192], mdt, tag="B")
        nc.scalar.dma_start(sb_xf[0:3, :], mcast(allf[0:3, XF:XF + 8192]))
        nc.sync.dma_start(sb_xf[64:67, :], mcast(allf[64:67, XF:XF + 8192]))
        sb_w0rr = wgt.tile([128, 64], mdt)
        nc.scalar.dma_start(sb_w0rr[:], mcast(allf[:, W0R:W0R + 64]))
        sb_w1t2r = wgt.tile([128, 128], mdt)
        nc.scalar.dma_start(sb_w1t2r[:], mcast(allf[:, W1T2:W1T2 + 128]))

        sb_eps = wgt.tile([128, 1], f32)
        nc.vector.memset(sb_eps[:], EPS)
        sb_ones = wgt.tile([128, 1], f32)
        nc.vector.memset(sb_ones[:], 1.0)

        def bn_affine(red, gbe, p, tag):
            """red [p,2] = (sumsq, sum) global; gbe [p,2] = (gamma, beta).
            Returns (scale, shift) [p,2]: y = scale*u + shift."""
            w = stat.tile([p, 6], f32, tag=f"bnw_{tag}")
            nc.scalar.mul(w[:, 0:2], red[:, 0:2], 1.0 / NTOT)  # E[u^2], mean
            nc.vector.tensor_mul(w[:, 2:3], w[:, 1:2], w[:, 1:2])
            nc.vector.tensor_sub(w[:, 3:4], w[:, 0:1], w[:, 2:3])  # var
            nc.scalar.activation(w[:, 4:5], w[:, 3:4], AF.Sqrt, bias=sb_eps[:p, 0:1])
            nc.vector.reciprocal(w[:, 5:6], w[:, 4:5])  # rstd
            sc = stat.tile([p, 2], f32, tag=f"bnsc_{tag}")
            nc.vector.tensor_mul(sc[:, 0:1], gbe[:, 0:1], w[:, 5:6])
            nc.vector.tensor_mul(sc[:, 1:2], w[:, 1:2], sc[:, 0:1])
            nc.vector.tensor_sub(sc[:, 1:2], gbe[:, 1:2], sc[:, 1:2])
            return sc

        def allreduce(src_ap, p, f, tag):
            d_in = dram.tile([p, f], f32, tag=f"ar_in_{tag}")
            d_out = nc.dram_tensor(
                f"cc_out_{tag}", [p, f], f32, kind="Internal", addr_space="Shared"
            )
            nc.gpsimd.dma_start(d_in[:], src_ap)
            nc.gpsimd.collective_compute(
                "AllReduce",
                OP.add,
                replica_groups=RG,
                ins=[d_in[:].opt()],
                outs=[d_out.ap().opt()],
            )
            red = stat.tile([p, f], f32, tag=f"ar_red_{tag}")
            nc.gpsimd.dma_start(red[:], d_out.ap())
            return red

        if STAGE < 2:
            dummy = stat.tile([128, 1], f32, tag="dummy")
            nc.vector.memset(dummy[:], 0.0)
            nc.sync.dma_start(out[0:128, 0:1], dummy[:])
        u0 = big.tile([128, 8192], mdt, tag="C")
        u1b = big.tile([128, N], bf16, tag="E")
        ar1 = stat.tile([128, 2], f32)

        with (
            tc.tile_pool(name="psE", bufs=2, space="PSUM") as psE,
            tc.tile_pool(name="psA", bufs=3, space="PSUM") as psA,
        ):
            # ------------ layer-0 global stats from replicated x ------------
            if STAGE < 1.2:
                return
            P = stat.tile([128, 9], f32)
            pairs = [(0, 0), (1, 1), (2, 2), (0, 1), (0, 2), (1, 2)]
            psc = scr.tile([128, 1024], f32, tag="sq")
            for i, (a, b) in enumerate(pairs):
                nc.vector.tensor_mul(
                    psc[:], sb_xall[:, a * 1024:(a + 1) * 1024],
                    sb_xall[:, b * 1024:(b + 1) * 1024])
                nc.vector.reduce_sum(P[:, i:i + 1], psc[:], axis=AX.X)
            for a in range(3):
                nc.vector.reduce_sum(P[:, 6 + a:7 + a],
                                     sb_xall[:, a * 1024:(a + 1) * 1024], axis=AX.X)

            if STAGE < 1.5:
                return
            ps1 = psE.tile([1, 9], f32, tag="s")
            nc.tensor.matmul(ps1[:], sb_ones[:], P[:])          # col-reduce
            grow = stat.tile([1, 9], f32)
            nc.scalar.copy(grow[:], ps1[:])
            ps2 = psE.tile([9, 1], f32, tag="s")
            nc.tensor.matmul(ps2[:], grow[:], sb_ones[0:1, 0:1])  # row -> col
            gcol = stat.tile([9, 1], f32)
            nc.scalar.copy(gcol[:], ps2[:])
            ps3 = psE.tile([64, 2], f32, tag="s")
            nc.tensor.matmul(ps3[:, 0:1], sb_wpa, gcol[:])      # sumsq(u0)
            nc.tensor.matmul(ps3[:, 1:2], sb_wpb, gcol[:])      # sum(u0)
            raw0 = stat.tile([64, 2], f32)
            nc.scalar.copy(raw0[:], ps3[:])
            ps4 = psE.tile([128, 2], f32, tag="s")
            nc.tensor.matmul(ps4[:], sb_irep, raw0[:])          # replicate
            raw0r = stat.tile([128, 2], f32)
            nc.scalar.copy(raw0r[:], ps4[:])
            sc0 = bn_affine(raw0r, sb_g0, 128, "bn0")
            if STAGE < 2:
                return

            # ---------------- layer 0 matmuls (folded PSUM) ----------------
            # u0 folded [128, 8192]: col 1024k+m <-> point 2048k+m
            # (partitions 0:64) / 2048k+1024+m (partitions 64:128).
            for k in range(8):
                pa = psA.tile([128, 1024], f32, tag="mm")
                tb = (k // 4) * 64  # xf band (8192 points each)
                base = (k % 4) * 2048
                for h in range(2):
                    for q in range(2):
                        nc.tensor.matmul(
                            pa[h * 64:(h + 1) * 64, q * 512:(q + 1) * 512],
                            sb_w0r[tb:tb + 3, :],
                            sb_xf[tb:tb + 3,
                                  base + h * 1024 + q * 512:
                                  base + h * 1024 + (q + 1) * 512].bitcast(f32),
                        )
                nc.scalar.copy(u0[:, k * 1024:(k + 1) * 1024], pa[:])

            # relu0 (folded, in place): h0 = relu(scale*u0 + shift), f32
            sh0c = stat.tile([128, 2], f32)
            for c in range(2):
                nc.scalar.activation(
                    u0[:, c * 4096:(c + 1) * 4096], u0[:, c * 4096:(c + 1) * 4096],
                    AF.Relu, bias=sc0[:, 1:2], scale=sc0[:, 0:1],
                    accum_out=sh0c[:, c:c + 1],
                )
            h0 = u0
            sh0 = stat.tile([128, 1], f32)
            nc.vector.tensor_add(sh0[:], sh0c[:, 0:1], sh0c[:, 1:2])

            # local features -> out rows 1024:1088 (f32, gpsimd, emitted
            # BEFORE the collectives to stay ahead in the Pool FIFO).
            h0f = h0[:].bitcast(f32) if F32R else h0[:]
            if LOCAL_SIMPLE:
                nc.gpsimd.dma_start(out[1024:1088, 0:8192], h0f[0:64, :])
                nc.gpsimd.dma_start(out[1024:1088, 8192:16384], h0f[64:128, :])
            else:
                dtop = out[1024:1088, :].rearrange("p (k m) -> p k m", k=8)
                nc.gpsimd.dma_start(
                    dtop[:, :, 0:1024],
                    h0f[0:64, :].rearrange("p (k m) -> p k m", k=8))
                nc.gpsimd.dma_start(
                    dtop[:, :, 1024:2048],
                    h0f[64:128, :].rearrange("p (k m) -> p k m", k=8))

            if STAGE < 3:
                return
            # ---------------- layer 1 ----------------
            q1p = stat.tile([128, 16], f32)
            for t in range(16):
                pb = psA.tile([128, 1024], f32, tag="mm")
                k = t // 2
                hb = (t % 2) * 64
                for q in range(2):
                    nc.tensor.matmul(
                        pb[:, q * 512:(q + 1) * 512],
                        sb_w1t2r[hb:hb + 64, :],
                        h0[hb:hb + 64,
                           k * 1024 + q * 512:k * 1024 + (q + 1) * 512],
                    )
                nc.scalar.copy(u1b[:, t * 1024:(t + 1) * 1024], pb[:])
                sq = scr.tile([128, 1024], bf16, tag="sq")
                nc.vector.tensor_mul(sq[:], u1b[:, t * 1024:(t + 1) * 1024],
                                     u1b[:, t * 1024:(t + 1) * 1024])
                nc.vector.reduce_sum(q1p[:, t:t + 1], sq[:], axis=AX.X)
            nc.vector.reduce_sum(ar1[:, 0:1], q1p[:], axis=AX.X)
            ps5 = psE.tile([128, 1], f32, tag="s")
            nc.tensor.matmul(ps5[:], sb_w1t2[:], sh0[:])     # s1 = W1 @ sum(h0)
            nc.scalar.copy(ar1[:, 1:2], ps5[:])

        if STAGE < 3:
            return
        red1 = allreduce(ar1[:], 128, 2, "bn1")

        # ---------------- post-AR1: BN1 affine, h1 + h1T + G ----------------
        sc1 = bn_affine(red1, sb_g1, 128, "bn1")
        if STAGE < 4:
            return
        h1 = big.tile([128, N], bf16, tag="C")
        h1T = big.tile([128, N], bf16, tag="B")
        sh1p = stat.tile([128, 8], f32)
        ar2 = stat.tile([128, 129], f32)
        with tc.tile_pool(name="psW", bufs=1, space="PSUM") as psW:
            pg = psW.tile([128, 128], f32, tag="g")
            for c in range(8):
                nc.scalar.activation(
                    h1[:, c * 2048:(c + 1) * 2048], u1b[:, c * 2048:(c + 1) * 2048],
                    AF.Relu, bias=sc1[:, 1:2], scale=sc1[:, 0:1],
                    accum_out=sh1p[:, c:c + 1],
                )
                if NO_TRANSPOSE:
                    continue
                for half in range(2):
                    ptp = psW.tile([128, 1024], bf16, tag="tp")
                    for s in range(8):
                        i = c * 16 + half * 8 + s
                        nc.tensor.transpose(
                            ptp[:, s * 128:(s + 1) * 128],
                            h1[:, i * 128:(i + 1) * 128], sb_i128b)
                    cb = (c * 2 + half) * 1024
                    nc.vector.tensor_copy(h1T[:, cb:cb + 1024], ptp[:])
                    for s in range(8):
                        i = c * 16 + half * 8 + s
                        nc.tensor.matmul(
                            pg[:], h1T[:, i * 128:(i + 1) * 128],
                            h1T[:, i * 128:(i + 1) * 128],
                            start=(i == 0), stop=(i == 127),
                        )
            if NO_TRANSPOSE:
                nc.vector.memset(ar2[:, 0:128], 1.0)
            else:
                nc.scalar.copy(ar2[:, 0:128], pg[:])
            nc.vector.reduce_sum(ar2[:, 128:129], sh1p[:], axis=AX.X)

        red2 = allreduce(ar2[:], 128, 129, "bn2")
        if STAGE < 5:
            return

        # ---------------- layer 2 blocks (overlap AR2) ----------------
        mx8 = stat.tile([128, 8], f32)
        with tc.tile_pool(name="psB", bufs=2, space="PSUM") as psB:
            for j in range(8):
                mxp = stat.tile([128, 8], f32, tag="mxp")
                for t in range(8):
                    pc = psB.tile([128, 2048], f32, tag="mm")
                    for q in range(4):
                        nc.tensor.matmul(
                            pc[:, q * 512:(q + 1) * 512],
                            sb_w2tb[:, j * 128:(j + 1) * 128],
                            h1[:, t * 2048 + q * 512:t * 2048 + (q + 1) * 512],
                        )
                    nc.vector.reduce_max(mxp[:, t:t + 1], pc[:], axis=AX.X)
                nc.vector.reduce_max(mx8[:, j:j + 1], mxp[:], axis=AX.X)

            # ------------- post-AR2: BN2 affine + gfeat -------------
            g2b = stat.tile([128, 128], bf16, tag="g2b")
            nc.vector.tensor_copy(g2b[:], red2[:, 0:128])
            sh1b = stat.tile([128, 1], bf16, tag="sh1b")
            nc.vector.tensor_copy(sh1b[:], red2[:, 128:129])
            q2s = stat.tile([128, 8], f32)
            m2 = stat.tile([128, 8], f32)
            sqs = scr.tile([128, 128], f32, tag="sqs")
            for j in range(8):
                pp = psB.tile([128, 2048], f32, tag="mm")
                nc.tensor.matmul(pp[:, 0:128],
                                 sb_w2tb[:, j * 128:(j + 1) * 128], g2b[:])
                nc.vector.tensor_mul(sqs[:], pp[:, 0:128],
                                     sb_w2n[:, j * 128:(j + 1) * 128])
                nc.vector.reduce_sum(q2s[:, j:j + 1], sqs[:], axis=AX.X)
                nc.tensor.matmul(pp[:, 128:129],
                                 sb_w2tb[:, j * 128:(j + 1) * 128], sh1b[:])
                nc.scalar.mul(m2[:, j:j + 1], pp[:, 128:129], 1.0 / NTOT)

            w2s = stat.tile([128, 8 * 4], f32)
            e2 = w2s[:, 0:8]
            m2sq = w2s[:, 8:16]
            var = w2s[:, 16:24]
            rstd = w2s[:, 24:32]
            nc.scalar.mul(e2, q2s[:], 1.0 / NTOT)
            nc.vector.tensor_mul(m2sq, m2[:], m2[:])
            nc.vector.tensor_sub(var, e2, m2sq)
            nc.scalar.activation(var, var, AF.Sqrt, bias=sb_eps[:, 0:1])
            nc.vector.reciprocal(rstd, var)
            sc2 = stat.tile([128, 8], f32)
            sh2 = stat.tile([128, 8], f32)
            nc.vector.tensor_mul(sc2[:], sb_g2[:], rstd)
            nc.vector.tensor_mul(sh2[:], m2[:], sc2[:])
            nc.vector.tensor_sub(sh2[:], sb_be2[:], sh2[:])
            gf = stat.tile([128, 8], f32)
            nc.vector.tensor_mul(gf[:], sc2[:], mx8[:])
            nc.vector.tensor_add(gf[:], gf[:], sh2[:])

            # broadcast writes: stage [128, 2048], DMA reads it 8x
            for j in range(8):
                bc = bcp.tile([128, 2048], f32, tag="bc")
                if BC_ON_DVE:
                    nc.vector.tensor_copy(bc[:], gf[:, j:j + 1].to_broadcast([128, 2048]))
                else:
                    nc.scalar.copy(bc[:], gf[:, j:j + 1].to_broadcast([128, 2048]))
                src = bc[:].unsqueeze(1).broadcast_to([128, 8, 2048])
                nc.sync.dma_start(out[j * 128:(j + 1) * 128, :], src)


@functools.lru_cache(maxsize=1)
def build_program():
    import concourse.bacc as bacc
    import concourse.tile as tile
    from concourse import mybir

    f32 = mybir.dt.float32
    nc = bacc.Bacc(
        "TRN2", target_bir_lowering=False, debug=False, num_devices=NCORES
    )
    io = {
        "allf": nc.dram_tensor("allf", [128, WTOT], f32, kind="ExternalInput").ap(),
        "out": nc.dram_tensor("out", [1088, N], f32, kind="ExternalOutput").ap(),
    }
    with tile.TileContext(nc) as tc:
        _body(nc, tc, io)
    nc.compile()
    return nc


def _to_bf16(a):
    import ml_dtypes
    return np.asarray(a, np.float32).astype(ml_dtypes.bfloat16)


def _bfpack(a):
    """[p, 2k] bf16 -> [p, k] f32 words."""
    return np.ascontiguousarray(_to_bf16(a)).view(np.float32)


def make_in_maps(x, W0, W1, W2, g0, be0, g1, be1, g2, be2):
    x = np.asarray(x, np.float32)
    W0 = np.asarray(W0, np.float32)
    W1 = np.asarray(W1, np.float32)
    W2 = np.asarray(W2, np.float32)

    base = np.zeros((128, WTOT), np.float32)
    # xall: component-major, all batches; point (q, p) -> flat index q*128+p
    xq = x.reshape(NTOT, 3).reshape(1024, 128, 3)
    for a in range(3):
        base[:, XALL + a * 1024:XALL + (a + 1) * 1024] = xq[:, :, a].T
    base[0:64, W1T2:W1T2 + 128] = W1.T
    base[64:128, W1T2:W1T2 + 128] = W1.T
    base[:, I128B:I128B + 64] = _bfpack(np.eye(128, dtype=np.float32))
    base[:, W2TB:W2TB + 512] = _bfpack(W2.T)         # [128, 1024] bf16
    w2n = np.zeros((128, 1024), np.float32)
    for j in range(8):
        # w2n[o, j*128 + c] = W2[j*128 + o, c]
        w2n[:, j * 128:(j + 1) * 128] = W2[j * 128:(j + 1) * 128, :]
    base[:, W2N:W2N + 1024] = w2n
    base[0:3, W0R:W0R + 64] = W0.T
    base[64:67, W0R:W0R + 64] = W0.T
    wpa = np.zeros((9, 64), np.float32)
    wpa[0] = W0[:, 0] * W0[:, 0]
    wpa[1] = W0[:, 1] * W0[:, 1]
    wpa[2] = W0[:, 2] * W0[:, 2]
    wpa[3] = 2 * W0[:, 0] * W0[:, 1]
    wpa[4] = 2 * W0[:, 0] * W0[:, 2]
    wpa[5] = 2 * W0[:, 1] * W0[:, 2]
    base[0:9, WPA:WPA + 64] = wpa
    wpb = np.zeros((9, 64), np.float32)
    wpb[6] = W0[:, 0]
    wpb[7] = W0[:, 1]
    wpb[8] = W0[:, 2]
    base[0:9, WPB:WPB + 64] = wpb
    base[0:64, IREP:IREP + 64] = np.eye(64, dtype=np.float32)
    base[0:64, IREP + 64:IREP + 128] = np.eye(64, dtype=np.float32)
    base[0:64, SMALL] = np.asarray(g0, np.float32)
    base[64:128, SMALL] = np.asarray(g0, np.float32)
    base[0:64, SMALL + 1] = np.asarray(be0, np.float32)
    base[64:128, SMALL + 1] = np.asarray(be0, np.float32)
    base[:, SMALL + 2] = np.asarray(g1, np.float32)
    base[:, SMALL + 3] = np.asarray(be1, np.float32)
    base[:, SMALL + 4:SMALL + 12] = np.asarray(g2, np.float32).reshape(8, 128).T
    base[:, SMALL + 12:SMALL + 20] = np.asarray(be2, np.float32).reshape(8, 128).T

    maps = []
    for i in range(NCORES):
        allf = base.copy()
        allf[0:3, XF:XF + 8192] = x[i, 0:8192, :].T
        allf[64:67, XF:XF + 8192] = x[i, 8192:16384, :].T
        maps.append({"allf": allf})
    return maps


def kernel(x, W0, b0, g0, be0, W1, b1, g1, be1, W2, b2, g2, be2):
    """Full inputs in, full output out.  b0/b1/b2 cancel inside BN."""
    from concourse.bass_utils import run_bass_kernel_spmd

    nc = build_program()
    in_maps = make_in_maps(x, W0, W1, W2, g0, be0, g1, be1, g2, be2)
    res = run_bass_kernel_spmd(nc, in_maps, core_ids=list(range(NCORES)))
    return np.stack([res.results[i]["out"] for i in range(NCORES)], axis=0)
